# revision 12
# baseline (speedup 1.0000x reference)
"""Contextual-attention Trainium2 kernel (Bass/Tile), work-balanced across cores.

Math (per sequence b):
    Q = evo @ q_w.T + q_b                                  (L, 96)
    K = cat(evo, conv3(evo), conv5(evo)) @ k_w.T + k_b     (L, 96)
    V = plm @ v_w.T + v_b                                  (L, 96)
    P = softmax(Q K^T / sqrt(96), key-masked by seqlen)
    out = P @ V + V

Design notes (measured on hw):
  * PE streams 1 output column/cycle (~1.35 GHz) regardless of dtype; fp8
    DoubleRow processes 2 k-tiles per pass -> 2x for pairable contractions
    (QT/KT/OT). The 96-dim score contraction is not pairable beyond a
    zero-padded 2x64 split, so ST cost is fixed; balancing it across cores
    is what wins.
  * Work per sequence scales with nkt_b = ceil(seqlen_b/128) key tiles
    (2..14 here). Attention is split into 4-key-tile "windows" (512 key
    cols x all 2048 queries); every core gets exactly wpc windows + 4 fp16
    V chunks -> perfectly uniform SPMD program; host sums the partial
    (numerator|denominator) window outputs.
  * The PE drops to a ~1.66x slower p-state after any blocking wait and
    needs ~3us of uninterrupted issue to recover, so the emission order
    keeps every PE instruction's deps satisfied ahead of time: VT chunks
    2..3 are interleaved as filler between ACT-bound score units, OT
    drains per half-window, DMA issue is spread across sync/gpsimd
    queues, and the scalar queue carries nothing but the exp ACTs.
  * Masking: per (window-tile) per-partition bias 0/-1e6 into the exp ACT
    reproduces the reference where()+softmax exactly (exp(-1e6+s) == 0).
"""

import os
import numpy as np
import ml_dtypes

import concourse.bacc as bacc
import concourse.bass as bass
import concourse.tile as tile
from concourse import mybir
from concourse._compat import get_trn_type
from concourse.bass_utils import run_bass_kernel_spmd

B, L = 8, 2048
Q_IN, V_IN, QK, VD = 512, 1024, 96, 96
P = 128
CW = 512          # column chunk width (= one PSUM bank of f32)
WT = 4            # key tiles per window
NCHUNK = L // CW  # 4 column chunks per sequence
NORM = float(1.0 / np.sqrt(QK))
F32 = mybir.dt.float32
F16 = mybir.dt.float16
F8 = mybir.dt.float8e4
NP8 = ml_dtypes.float8_e4m3
DR = mybir.MatmulPerfMode.DoubleRow
EXP = mybir.ActivationFunctionType.Exp

LAST_EXEC_TIME_NS = None
LAST_RESULTS = None

_program_cache = {}


def _fold_k_weights(k_w, k_b, cn3_w, cn3_b, cn5_w, cn5_b):
    """K[l] = sum_{t in -2..2} evo[l+t] @ taps[t+2] + bk  (zero-padded shifts)."""
    A_evo = k_w[:, :Q_IN]
    A3 = k_w[:, Q_IN : Q_IN + VD]
    A5 = k_w[:, Q_IN + VD :]
    taps = np.zeros((5, Q_IN, QK), np.float32)
    for j in range(3):  # conv3 tap j acts at offset t = j-1
        taps[j - 1 + 2] += np.einsum("oc,cd->do", A3, cn3_w[:, :, j]).astype(np.float32)
    for j in range(5):  # conv5 tap j acts at offset t = j-2
        taps[j - 2 + 2] += np.einsum("oc,cd->do", A5, cn5_w[:, :, j]).astype(np.float32)
    taps[2] += A_evo.T
    bk = (k_b + A3 @ cn3_b + A5 @ cn5_b).astype(np.float32)
    return taps, bk


def _build_program(wpc):
    nc = bacc.Bacc(get_trn_type() or "TRN2", target_bir_lowering=False, debug=False)
    wq = nc.declare_dram_parameter("wq", [P, 2 * 2 * P], F8, isOutput=False)
    wk = nc.declare_dram_parameter("wk", [P, 5 * 2 * 2 * P], F8, isOutput=False)
    wv = nc.declare_dram_parameter("wv", [P, 8 * QK], F16, isOutput=False)
    bqk = nc.declare_dram_parameter("bqk", [P, 2], F32, isOutput=False)
    bv = nc.declare_dram_parameter("bv", [QK, 1], F32, isOutput=False)
    maskd = nc.declare_dram_parameter("mask", [P, wpc * WT], F32, isOutput=False)
    identd = nc.declare_dram_parameter("ident", [QK, QK], F16, isOutput=False)
    evoW = [
        nc.declare_dram_parameter(f"evoW{w}", [P, 4 * (CW + 4)], F8, isOutput=False)
        for w in range(wpc)
    ]
    evoQ = [
        nc.declare_dram_parameter(f"evoQ{w}", [P, 4 * L], F8, isOutput=False)
        for w in range(wpc)
    ]
    plm = [
        nc.declare_dram_parameter(f"plm{r}", [P, 8 * CW], F16, isOutput=False)
        for r in range(NCHUNK)
    ]
    ot_out = [
        nc.declare_dram_parameter(f"ot{w}", [QK + 1, L], F32, isOutput=True)
        for w in range(wpc)
    ]
    vt_out = [
        nc.declare_dram_parameter(f"vt{r}", [QK, CW], F16, isOutput=True)
        for r in range(NCHUNK)
    ]

    add = mybir.AluOpType.add

    with tile.TileContext(nc) as tc:
        with tc.tile_pool(name="sing", bufs=1) as sing:
            # ---- SBUF tiles ----
            wq_sb = sing.tile([P, 2, 2, P], F8, tag="wq")
            wk_sb = sing.tile([P, 5, 2, 2, P], F8, tag="wk")
            wv_sb = sing.tile([P, 8, QK], F16, tag="wv")
            bqk_sb = sing.tile([P, 2], F32, tag="bqk")
            bv_sb = sing.tile([QK, 1], F32, tag="bv")
            mask_sb = sing.tile([P, wpc * WT], F32, tag="mask")
            ident_sb = sing.tile([QK, QK], F16, tag="ident")
            evw_sb = [
                [sing.tile([P, 2, CW + 4], F8, tag=f"evw{w}_{g}", name=f"evw{w}_{g}")
                 for g in range(2)]
                for w in range(wpc)
            ]
            evq_sb = [
                [sing.tile([P, 2, L], F8, tag=f"evq{w}_{g}", name=f"evq{w}_{g}")
                 for g in range(2)]
                for w in range(wpc)
            ]
            plm_sb = [
                sing.tile([P, 8, CW], F16, tag=f"plm{r}", name=f"plmsb{r}")
                for r in range(NCHUNK)
            ]
            kt_sb = [sing.tile([64, 2, CW], F8, tag=f"kt{w}", name=f"kt{w}") for w in range(wpc)]
            qt_sb = [sing.tile([64, 2, L], F8, tag=f"qt{w}", name=f"qt{w}") for w in range(wpc)]
            vt_sb = [sing.tile([QK, CW], F16, tag=f"vt{r}", name=f"vt{r}") for r in range(NCHUNK)]
            v1_sb = [sing.tile([P, 2, 2, 112], F8, tag=f"v1_{w}", name=f"v1_{w}") for w in range(wpc)]
            et_sb = [sing.tile([P, 2, 2, L], F8, tag=f"et{w}", name=f"et{w}") for w in range(wpc)]
            ot_sb = [sing.tile([QK + 1, L], F32, tag=f"ot{w}", name=f"ot{w}") for w in range(wpc)]

            # ---- DMA issue. Priority: small KT/QT-critical loads first on
            # all queues; the 4MB plm flood is split x8 and spread across
            # sync/vector/gpsimd so early engine-queues stay short. ----
            # sync: wq, wk, bqk, mask, evq-w1, plm r3 (+ OT outs later)
            nc.sync.dma_start(
                out=wq_sb, in_=wq[:, :].rearrange("p (g i m) -> p g i m", g=2, i=2)
            )
            wk_r = wk[:, :].rearrange("p (t g i m) -> p t g i m", t=5, g=2, i=2)
            for t4 in range(5):
                nc.sync.dma_start(out=wk_sb[:, t4], in_=wk_r[:, t4])
            nc.sync.dma_start(out=bqk_sb, in_=bqk[:, :])
            nc.sync.dma_start(out=mask_sb, in_=maskd[:, :])
            # gpsimd: evw (KT-critical), ident, wv, bv, then plm r1/r2
            for w in range(wpc):
                for g in range(2):
                    nc.gpsimd.dma_start(
                        out=evw_sb[w][g],
                        in_=evoW[w][:, :].rearrange("p (g i c) -> p g i c", g=2, i=2)[:, g],
                    )
            nc.gpsimd.dma_start(out=ident_sb, in_=identd[:, :])
            nc.gpsimd.dma_start(
                out=wv_sb, in_=wv[:, :].rearrange("p (n o) -> p n o", o=QK)
            )
            nc.gpsimd.dma_start(out=bv_sb, in_=bv[:, :])
            # scalar: evq-w0 (QT-critical), then plm r0 (scalar is idle early)
            for g in range(2):
                for h in range(2):
                    nc.scalar.dma_start(
                        out=evq_sb[0][g][h * 64 : (h + 1) * 64],
                        in_=evoQ[0][h * 64 : (h + 1) * 64, :]
                        .rearrange("p (g i c) -> p g i c", g=2, i=2)[:, g],
                    )
            for w in range(1, wpc):
                for g in range(2):
                    for h in range(2):
                        nc.sync.dma_start(
                            out=evq_sb[w][g][h * 64 : (h + 1) * 64],
                            in_=evoQ[w][h * 64 : (h + 1) * 64, :]
                            .rearrange("p (g i c) -> p g i c", g=2, i=2)[:, g],
                        )

            def plm_issue(eng, r):
                for n in range(8):
                    eng.dma_start(
                        out=plm_sb[r][:, n : n + 1, :],
                        in_=plm[r][:, :].rearrange("p (n c) -> p n c", n=8)[
                            :, n : n + 1
                        ],
                    )

            plm_issue(nc.scalar, 0)
            if NCHUNK > 1:
                plm_issue(nc.gpsimd, 1)
            if NCHUNK > 2:
                plm_issue(nc.gpsimd, 2)
            if NCHUNK > 3:
                plm_issue(nc.sync, 3)

            for w in range(wpc):
                nc.vector.memset(v1_sb[w], 0.0)
                for p in range(2):
                    for i in range(2):
                        nc.vector.memset(v1_sb[w][:, p, i, QK : QK + 1], 1.0)

            IDENT = mybir.ActivationFunctionType.Identity

            def kt_split(w, pt):
                nc.vector.tensor_scalar(
                    out=kt_sb[w][:, 0, :], in0=pt[0:64, :],
                    scalar1=bqk_sb[0:64, 1:2], scalar2=None, op0=add,
                )
                nc.scalar.activation(
                    out=kt_sb[w][:, 1, :], in_=pt[64:128, :],
                    func=IDENT, bias=bqk_sb[64:128, 1:2], scale=1.0,
                )

            def qt_split(w, c, pt):
                nc.vector.tensor_scalar(
                    out=qt_sb[w][:, 0, c * CW : (c + 1) * CW], in0=pt[0:64, :],
                    scalar1=bqk_sb[0:64, 0:1], scalar2=None, op0=add,
                )
                nc.scalar.activation(
                    out=qt_sb[w][:, 1, c * CW : (c + 1) * CW], in_=pt[64:128, :],
                    func=IDENT, bias=bqk_sb[64:128, 0:1], scale=1.0,
                )

            with tc.tile_pool(name="v_psum", bufs=2, space="PSUM") as v_psum:

                def vt_head(r):
                    # returns the psum tile; 8 accumulation matmuls issued via vt_mm
                    return v_psum.tile([QK, CW], F32, tag="v", name=f"vtp{r}")

                def vt_mm(pt, r, dt):
                    nc.tensor.matmul(
                        pt, lhsT=wv_sb[:, dt], rhs=plm_sb[r][:, dt],
                        start=(dt == 0), stop=(dt == 7),
                    )

                def vt_drain(pt, r):
                    nc.vector.tensor_scalar(
                        out=vt_sb[r], in0=pt, scalar1=bv_sb[:, 0:1],
                        scalar2=None, op0=add,
                    )
                    nc.gpsimd.dma_start(out=vt_out[r][:, :], in_=vt_sb[r])

                # ---- projections: KT, QT (DoubleRow fp8), VT chunks 0..wpc-1 ----
                with (
                    tc.tile_pool(name="kq_psum", bufs=4, space="PSUM") as kq_psum,
                    tc.tile_pool(name="t_psum", bufs=2, space="PSUM") as t_psum,
                ):
                    for w in range(wpc):
                        pt = kq_psum.tile([P, CW], F32, tag="kq")
                        n = 0
                        for t in range(5):
                            for g in range(2):
                                nc.tensor.matmul(
                                    pt,
                                    lhsT=wk_sb[:, t, g],
                                    rhs=evw_sb[w][g][:, :, t : t + CW],
                                    start=(n == 0),
                                    stop=(n == 9),
                                    perf_mode=DR,
                                )
                                n += 1
                        kt_split(w, pt)
                    for w in range(wpc):
                        for c in range(NCHUNK):
                            pt = kq_psum.tile([P, CW], F32, tag="kq")
                            for g in range(2):
                                nc.tensor.matmul(
                                    pt,
                                    lhsT=wq_sb[:, g],
                                    rhs=evq_sb[w][g][:, :, c * CW : (c + 1) * CW],
                                    start=(g == 0),
                                    stop=(g == 1),
                                    perf_mode=DR,
                                )
                            qt_split(w, c, pt)
                    # VT chunks 0..wpc-1 (feed V1) + their transposes
                    for r in range(min(wpc, NCHUNK)):
                        pt = vt_head(r)
                        for dt in range(8):
                            vt_mm(pt, r, dt)
                        vt_drain(pt, r)
                        for j in range(WT):
                            vp = t_psum.tile([P, QK], F16, tag="t")
                            nc.tensor.transpose(
                                vp, vt_sb[r][:, j * P : (j + 1) * P], ident_sb
                            )
                            nc.vector.tensor_copy(
                                out=v1_sb[r][:, j // 2, j % 2, :QK], in_=vp
                            )

                # ---- attention. Scalar exp ACTs pace the pipeline; between
                # every score unit the PE gets 2 filler matmuls (VT chunks
                # wpc..3 and dribbled OT accumulations) so it never idles
                # and holds max p-state. ----
                fills = []
                for r in range(wpc, NCHUNK):
                    holder = {}

                    def mk(r, dt, holder):
                        def run():
                            if dt == 0:
                                holder["pt"] = vt_head(r)
                            vt_mm(holder["pt"], r, dt)
                            if dt == 7:
                                vt_drain(holder["pt"], r)
                        return run

                    for dt in range(8):
                        fills.append(mk(r, dt, holder))
                fills.reverse()  # pop() from the front order

                def fill_one():
                    if fills:
                        fills.pop()()

                with (
                    tc.tile_pool(name="st_psum", bufs=2, space="PSUM") as st_psum,
                    tc.tile_pool(name="ot_psum", bufs=1, space="PSUM") as ot_psum,
                ):
                    ot_tiles = {}

                    def aunit(w, j, h):
                        stp = st_psum.tile([P, 2 * CW], F32, tag="st")
                        for o in range(2):
                            nc.tensor.matmul(
                                stp[:, o * CW : (o + 1) * CW],
                                lhsT=kt_sb[w][:, :, j * P : (j + 1) * P],
                                rhs=qt_sb[w][
                                    :, :, (2 * h + o) * CW : (2 * h + o + 1) * CW
                                ],
                                start=True,
                                stop=True,
                                perf_mode=DR,
                            )
                        nc.scalar.activation(
                            out=et_sb[w][:, j // 2, j % 2, h * 2 * CW : (h + 1) * 2 * CW],
                            in_=stp,
                            func=EXP,
                            bias=mask_sb[:, w * WT + j : w * WT + j + 1],
                            scale=NORM,
                        )

                    def ot_mm(w, h, p):
                        # one OT pair-accumulation step (2 matmuls: chunks of half h)
                        if (w, h) not in ot_tiles:
                            ot_tiles[(w, h)] = ot_psum.tile(
                                [112, 2 * CW], F32, tag="ot", name=f"otp{w}_{h}"
                            )
                        otp = ot_tiles[(w, h)]
                        for o in range(2):
                            c = 2 * h + o
                            nc.tensor.matmul(
                                otp[:, o * CW : (o + 1) * CW],
                                lhsT=v1_sb[w][:, p],
                                rhs=et_sb[w][:, p, :, c * CW : (c + 1) * CW],
                                start=(p == 0),
                                stop=(p == 1),
                                perf_mode=DR,
                            )

                    def ot_drain(w, h):
                        otp = ot_tiles.pop((w, h))
                        nc.vector.tensor_copy(
                            out=ot_sb[w][:, h * 2 * CW : (h + 1) * 2 * CW],
                            in_=otp[: QK + 1, :],
                        )
                        for s in range(8):
                            c0 = h * 2 * CW + s * P
                            nc.sync.dma_start(
                                out=ot_out[w][:QK, c0 : c0 + P],
                                in_=ot_sb[w][:QK, c0 : c0 + P],
                            )
                        nc.gpsimd.dma_start(
                            out=ot_out[w][QK : QK + 1, h * 2 * CW : (h + 1) * 2 * CW],
                            in_=ot_sb[w][QK : QK + 1, h * 2 * CW : (h + 1) * 2 * CW],
                        )

                    # per-window unit stream (j,h) with slotted fillers:
                    #   u3 -> OT(h0,p0); u5 -> OT(h0,p1)+drain; u6 -> OT(h1,p0)
                    #   next window's u0 (or post-loop) -> OT(h1,p1)+drain
                    UNITS = [(0, 0), (1, 0), (2, 0), (3, 0), (0, 1), (1, 1), (2, 1), (3, 1)]
                    carry = None
                    for w in range(wpc):
                        for u, (j, h) in enumerate(UNITS):
                            aunit(w, j, h)
                            if u == 0 and carry is not None:
                                ot_mm(*carry, 1)
                                ot_drain(*carry)
                                carry = None
                            elif u == 3:
                                ot_mm(w, 0, 0)
                            elif u == 5:
                                ot_mm(w, 0, 1)
                                ot_drain(w, 0)
                            elif u == 6:
                                ot_mm(w, 1, 0)
                            else:
                                fill_one()
                                fill_one()
                        carry = (w, 1)
                    if carry is not None:
                        ot_mm(*carry, 1)
                        ot_drain(*carry)
                    while fills:
                        fills.pop()()
    nc.finalize()
    return nc


def _pack_pair_w(w, nk):
    """(nk*128, M) f32 -> [128, nk/2, 2, M] DoubleRow pair layout."""
    kt, m = nk, w.shape[1]
    v = w.reshape(kt, P, m).reshape(kt // 2, 2, P, m).transpose(2, 0, 1, 3)
    return np.ascontiguousarray(v)


def _plan(seqlengths):
    """Assign windows + residual chunks to cores."""
    nkt = [max(1, min(L // P, -(-int(s) // P))) for s in seqlengths]
    wins = [(b, w) for b in range(B) for w in range(-(-nkt[b] // WT))]
    wpc = max(1, -(-len(wins) // B))
    wins = wins + [None] * (B * wpc - len(wins))
    windows = [wins[c * wpc : (c + 1) * wpc] for c in range(B)]
    # R slot w must hold window w's key-column chunk (V1 derives locally)
    all_chunks = {(b, r) for b in range(B) for r in range(NCHUNK)}
    chunks = [[None] * NCHUNK for _ in range(B)]
    for c in range(B):
        for w, bw in enumerate(windows[c]):
            if w < NCHUNK and bw is not None:
                assert bw in all_chunks
                chunks[c][w] = bw
                all_chunks.discard(bw)
    rest = sorted(all_chunks)
    for c in range(B):
        for r in range(NCHUNK):
            if chunks[c][r] is None:
                chunks[c][r] = rest.pop()
    assert not rest
    return wpc, nkt, windows, chunks


def _prep_core(core, wpc, nkt, windows, chunks, evoT8, plmT, seqlengths, weights):
    m = dict(weights)
    mask = np.full((P, wpc * WT), -1e6, np.float32)
    p = np.arange(P)
    for w, bw in enumerate(windows[core]):
        if bw is None:
            m[f"evoW{w}"] = np.zeros((P, 4 * (CW + 4)), NP8)
            m[f"evoQ{w}"] = np.zeros((P, 4 * L), NP8)
            continue
        b, wi = bw
        sl = int(seqlengths[b])
        base = wi * WT * P
        for j in range(WT):
            mask[:, w * WT + j] = np.where(base + j * P + p < sl, 0.0, -1e6)
        sl_ = evoT8[b][:, base : base + CW + 4]
        m[f"evoW{w}"] = np.ascontiguousarray(
            sl_.reshape(4, P, CW + 4).transpose(1, 0, 2).reshape(P, -1)
        )
        m[f"evoQ{w}"] = np.ascontiguousarray(
            evoT8[b][:, 2 : 2 + L].reshape(4, P, L).transpose(1, 0, 2).reshape(P, -1)
        )
    m["mask"] = mask
    for r, (b, rc) in enumerate(chunks[core]):
        sl_ = plmT[b][:, rc * CW : (rc + 1) * CW]
        m[f"plm{r}"] = np.ascontiguousarray(
            sl_.reshape(8, P, CW).transpose(1, 0, 2).reshape(P, -1)
        )
    return m


def kernel(
    plm_embedding,
    evo_local,
    seqlengths,
    q_w,
    q_b,
    k_w,
    k_b,
    v_w,
    v_b,
    cn3_w,
    cn3_b,
    cn5_w,
    cn5_b,
):
    global LAST_EXEC_TIME_NS, LAST_RESULTS
    plm_embedding = np.asarray(plm_embedding, np.float32)
    evo_local = np.asarray(evo_local, np.float32)
    seqlengths = np.asarray(seqlengths)

    taps, bk = _fold_k_weights(
        np.asarray(k_w, np.float32),
        np.asarray(k_b, np.float32),
        np.asarray(cn3_w, np.float32),
        np.asarray(cn3_b, np.float32),
        np.asarray(cn5_w, np.float32),
        np.asarray(cn5_b, np.float32),
    )
    wpc, nkt, windows, chunks = _plan(seqlengths)

    # fp8 weights, M padded 96 -> 128 with zeros (pad rows of Q/K then
    # contribute exactly 0 to scores; biases pad with zeros too)
    wq_p = np.zeros((Q_IN, P), np.float32)
    wq_p[:, :QK] = np.asarray(q_w, np.float32).T
    wk_p = np.zeros((5 * Q_IN, P), np.float32)
    wk_p[:, :QK] = taps.reshape(5 * Q_IN, QK)
    bqk = np.zeros((P, 2), np.float32)
    bqk[:QK, 0] = np.asarray(q_b, np.float32)
    bqk[:QK, 1] = bk
    weights = {
        "wq": np.ascontiguousarray(_pack_pair_w(wq_p, 4).reshape(P, -1)).astype(NP8),
        "wk": np.ascontiguousarray(_pack_pair_w(wk_p, 20).reshape(P, -1)).astype(NP8),
        "wv": np.ascontiguousarray(
            np.asarray(v_w, np.float32)
            .T.reshape(8, P, QK)
            .transpose(1, 0, 2)
            .reshape(P, -1)
        ).astype(np.float16),
        "bqk": bqk,
        "bv": np.ascontiguousarray(np.asarray(v_b, np.float32)[:, None]),
        "ident": np.eye(QK, dtype=np.float16),
    }

    evoT8 = np.zeros((B, Q_IN, L + 4), NP8)
    evoT8[:, :, 2 : 2 + L] = np.clip(
        evo_local.transpose(0, 2, 1), -240.0, 240.0
    ).astype(NP8)
    plmT = plm_embedding.transpose(0, 2, 1).astype(np.float16)

    if wpc not in _program_cache:
        _program_cache[wpc] = _build_program(wpc)
    nc = _program_cache[wpc]

    in_maps = [
        _prep_core(c, wpc, nkt, windows, chunks, evoT8, plmT, seqlengths, weights)
        for c in range(B)
    ]
    trace = bool(os.environ.get("KBENCH_TRACE"))
    res = run_bass_kernel_spmd(nc, in_maps, list(range(B)), trace=trace)
    LAST_EXEC_TIME_NS = res.exec_time_ns
    LAST_RESULTS = res

    num = np.zeros((B, QK, L), np.float32)
    den = np.zeros((B, 1, L), np.float32)
    vt = np.zeros((B, QK, L), np.float32)
    for c in range(B):
        for w, bw in enumerate(windows[c]):
            if bw is None:
                continue
            b, _ = bw
            ot = res.results[c][f"ot{w}"]
            num[b] += ot[:QK]
            den[b] += ot[QK : QK + 1]
        for r, (b, rc) in enumerate(chunks[c]):
            vt[b][:, rc * CW : (rc + 1) * CW] = res.results[c][f"vt{r}"]
    out = ((num / den) + vt).transpose(0, 2, 1).astype(np.float32)
    return np.ascontiguousarray(out)


# revision 13
# speedup vs baseline: 1.0066x; 1.0066x over previous
"""Contextual-attention Trainium2 kernel (Bass/Tile), work-balanced across cores.

Math (per sequence b):
    Q = evo @ q_w.T + q_b                                  (L, 96)
    K = cat(evo, conv3(evo), conv5(evo)) @ k_w.T + k_b     (L, 96)
    V = plm @ v_w.T + v_b                                  (L, 96)
    P = softmax(Q K^T / sqrt(96), key-masked by seqlen)
    out = P @ V + V

Design notes (measured on hw):
  * PE streams 1 output column/cycle (~1.35 GHz) regardless of dtype; fp8
    DoubleRow processes 2 k-tiles per pass -> 2x for pairable contractions
    (QT/KT/OT). The 96-dim score contraction is not pairable beyond a
    zero-padded 2x64 split, so ST cost is fixed; balancing it across cores
    is what wins.
  * Work per sequence scales with nkt_b = ceil(seqlen_b/128) key tiles
    (2..14 here). Attention is split into 4-key-tile "windows" (512 key
    cols x all 2048 queries); every core gets exactly wpc windows + 4 fp16
    V chunks -> perfectly uniform SPMD program; host sums the partial
    (numerator|denominator) window outputs.
  * The PE drops to a ~1.66x slower p-state after any blocking wait and
    needs ~3us of uninterrupted issue to recover, so the emission order
    keeps every PE instruction's deps satisfied ahead of time: VT chunks
    2..3 are interleaved as filler between ACT-bound score units, OT
    drains per half-window, DMA issue is spread across sync/gpsimd
    queues, and the scalar queue carries nothing but the exp ACTs.
  * Masking: per (window-tile) per-partition bias 0/-1e6 into the exp ACT
    reproduces the reference where()+softmax exactly (exp(-1e6+s) == 0).
"""

import os
import numpy as np
import ml_dtypes

import concourse.bacc as bacc
import concourse.bass as bass
import concourse.tile as tile
from concourse import mybir
from concourse._compat import get_trn_type
from concourse.bass_utils import run_bass_kernel_spmd

B, L = 8, 2048
Q_IN, V_IN, QK, VD = 512, 1024, 96, 96
P = 128
CW = 512          # column chunk width (= one PSUM bank of f32)
WT = 4            # key tiles per window
NCHUNK = L // CW  # 4 column chunks per sequence
NORM = float(1.0 / np.sqrt(QK))
F32 = mybir.dt.float32
F16 = mybir.dt.float16
F8 = mybir.dt.float8e4
NP8 = ml_dtypes.float8_e4m3
DR = mybir.MatmulPerfMode.DoubleRow
EXP = mybir.ActivationFunctionType.Exp

LAST_EXEC_TIME_NS = None
LAST_RESULTS = None

_program_cache = {}


def _fold_k_weights(k_w, k_b, cn3_w, cn3_b, cn5_w, cn5_b):
    """K[l] = sum_{t in -2..2} evo[l+t] @ taps[t+2] + bk  (zero-padded shifts)."""
    A_evo = k_w[:, :Q_IN]
    A3 = k_w[:, Q_IN : Q_IN + VD]
    A5 = k_w[:, Q_IN + VD :]
    taps = np.zeros((5, Q_IN, QK), np.float32)
    for j in range(3):  # conv3 tap j acts at offset t = j-1
        taps[j - 1 + 2] += np.einsum("oc,cd->do", A3, cn3_w[:, :, j]).astype(np.float32)
    for j in range(5):  # conv5 tap j acts at offset t = j-2
        taps[j - 2 + 2] += np.einsum("oc,cd->do", A5, cn5_w[:, :, j]).astype(np.float32)
    taps[2] += A_evo.T
    bk = (k_b + A3 @ cn3_b + A5 @ cn5_b).astype(np.float32)
    return taps, bk


def _build_program(wpc):
    nc = bacc.Bacc(get_trn_type() or "TRN2", target_bir_lowering=False, debug=False)
    wq = nc.declare_dram_parameter("wq", [P, 2 * 2 * P], F8, isOutput=False)
    wk = nc.declare_dram_parameter("wk", [P, 5 * 2 * 2 * P], F8, isOutput=False)
    wv = nc.declare_dram_parameter("wv", [P, 8 * QK], F16, isOutput=False)
    bqk = nc.declare_dram_parameter("bqk", [P, 2], F32, isOutput=False)
    bv = nc.declare_dram_parameter("bv", [QK, 1], F32, isOutput=False)
    maskd = nc.declare_dram_parameter("mask", [P, wpc * WT], F32, isOutput=False)
    identd = nc.declare_dram_parameter("ident", [QK, QK], F16, isOutput=False)
    evoW = [
        nc.declare_dram_parameter(f"evoW{w}", [P, 4 * (CW + 4)], F8, isOutput=False)
        for w in range(wpc)
    ]
    evoQ = [
        nc.declare_dram_parameter(f"evoQ{w}", [P, 4 * L], F8, isOutput=False)
        for w in range(wpc)
    ]
    plm = [
        nc.declare_dram_parameter(f"plm{r}", [P, 8 * CW], F16, isOutput=False)
        for r in range(NCHUNK)
    ]
    ot_out = [
        nc.declare_dram_parameter(f"ot{w}", [QK + 1, L], F16, isOutput=True)
        for w in range(wpc)
    ]
    vt_out = [
        nc.declare_dram_parameter(f"vt{r}", [QK, CW], F16, isOutput=True)
        for r in range(NCHUNK)
    ]

    add = mybir.AluOpType.add

    with tile.TileContext(nc) as tc:
        with tc.tile_pool(name="sing", bufs=1) as sing:
            # ---- SBUF tiles ----
            wq_sb = sing.tile([P, 2, 2, P], F8, tag="wq")
            wk_sb = sing.tile([P, 5, 2, 2, P], F8, tag="wk")
            wv_sb = sing.tile([P, 8, QK], F16, tag="wv")
            bqk_sb = sing.tile([P, 2], F32, tag="bqk")
            bv_sb = sing.tile([QK, 1], F32, tag="bv")
            mask_sb = sing.tile([P, wpc * WT], F32, tag="mask")
            ident_sb = sing.tile([QK, QK], F16, tag="ident")
            evw_sb = [
                [sing.tile([P, 2, CW + 4], F8, tag=f"evw{w}_{g}", name=f"evw{w}_{g}")
                 for g in range(2)]
                for w in range(wpc)
            ]
            evq_sb = [
                [sing.tile([P, 2, L], F8, tag=f"evq{w}_{g}", name=f"evq{w}_{g}")
                 for g in range(2)]
                for w in range(wpc)
            ]
            plm_sb = [
                sing.tile([P, 8, CW], F16, tag=f"plm{r}", name=f"plmsb{r}")
                for r in range(NCHUNK)
            ]
            kt_sb = [sing.tile([64, 2, CW], F8, tag=f"kt{w}", name=f"kt{w}") for w in range(wpc)]
            qt_sb = [sing.tile([64, 2, L], F8, tag=f"qt{w}", name=f"qt{w}") for w in range(wpc)]
            vt_sb = [sing.tile([QK, CW], F16, tag=f"vt{r}", name=f"vt{r}") for r in range(NCHUNK)]
            v1_sb = [sing.tile([P, 2, 2, 112], F8, tag=f"v1_{w}", name=f"v1_{w}") for w in range(wpc)]
            et_sb = [sing.tile([P, 2, 2, L], F8, tag=f"et{w}", name=f"et{w}") for w in range(wpc)]
            ot_sb = [sing.tile([QK + 1, L], F16, tag=f"ot{w}", name=f"ot{w}") for w in range(wpc)]

            # ---- DMA issue. Priority: small KT/QT-critical loads first on
            # all queues; the 4MB plm flood is split x8 and spread across
            # sync/vector/gpsimd so early engine-queues stay short. ----
            # sync: wq, wk, bqk, mask, evq-w1, plm r3 (+ OT outs later)
            nc.sync.dma_start(
                out=wq_sb, in_=wq[:, :].rearrange("p (g i m) -> p g i m", g=2, i=2)
            )
            wk_r = wk[:, :].rearrange("p (t g i m) -> p t g i m", t=5, g=2, i=2)
            for t4 in range(5):
                nc.sync.dma_start(out=wk_sb[:, t4], in_=wk_r[:, t4])
            nc.sync.dma_start(out=bqk_sb, in_=bqk[:, :])
            nc.sync.dma_start(out=mask_sb, in_=maskd[:, :])
            # gpsimd: evw (KT-critical), ident, wv, bv, then plm r1/r2
            for w in range(wpc):
                for g in range(2):
                    nc.gpsimd.dma_start(
                        out=evw_sb[w][g],
                        in_=evoW[w][:, :].rearrange("p (g i c) -> p g i c", g=2, i=2)[:, g],
                    )
            nc.gpsimd.dma_start(out=ident_sb, in_=identd[:, :])
            nc.gpsimd.dma_start(
                out=wv_sb, in_=wv[:, :].rearrange("p (n o) -> p n o", o=QK)
            )
            nc.gpsimd.dma_start(out=bv_sb, in_=bv[:, :])
            # scalar: evq-w0 (QT-critical), then plm r0 (scalar is idle early)
            for g in range(2):
                for h in range(2):
                    for q in range(2):
                        nc.scalar.dma_start(
                            out=evq_sb[0][g][h * 64 : (h + 1) * 64, :, q * 1024 : (q + 1) * 1024],
                            in_=evoQ[0][h * 64 : (h + 1) * 64, :]
                            .rearrange("p (g i c) -> p g i c", g=2, i=2)[:, g, :, q * 1024 : (q + 1) * 1024],
                        )
            for w in range(1, wpc):
                for g in range(2):
                    for h in range(2):
                        for q in range(2):
                            nc.sync.dma_start(
                                out=evq_sb[w][g][h * 64 : (h + 1) * 64, :, q * 1024 : (q + 1) * 1024],
                                in_=evoQ[w][h * 64 : (h + 1) * 64, :]
                                .rearrange("p (g i c) -> p g i c", g=2, i=2)[:, g, :, q * 1024 : (q + 1) * 1024],
                            )

            def plm_issue(eng, r):
                for n in range(8):
                    eng.dma_start(
                        out=plm_sb[r][:, n : n + 1, :],
                        in_=plm[r][:, :].rearrange("p (n c) -> p n c", n=8)[
                            :, n : n + 1
                        ],
                    )

            plm_issue(nc.scalar, 0)
            if NCHUNK > 1:
                plm_issue(nc.gpsimd, 1)
            if NCHUNK > 2:
                plm_issue(nc.gpsimd, 2)
            if NCHUNK > 3:
                plm_issue(nc.sync, 3)

            for w in range(wpc):
                nc.vector.memset(v1_sb[w], 0.0)
                for p in range(2):
                    for i in range(2):
                        nc.vector.memset(v1_sb[w][:, p, i, QK : QK + 1], 1.0)

            IDENT = mybir.ActivationFunctionType.Identity

            def kt_split(w, pt):
                nc.vector.tensor_scalar(
                    out=kt_sb[w][:, 0, :], in0=pt[0:64, :],
                    scalar1=bqk_sb[0:64, 1:2], scalar2=None, op0=add,
                )
                nc.scalar.activation(
                    out=kt_sb[w][:, 1, :], in_=pt[64:128, :],
                    func=IDENT, bias=bqk_sb[64:128, 1:2], scale=1.0,
                )

            def qt_split(w, c, pt):
                nc.vector.tensor_scalar(
                    out=qt_sb[w][:, 0, c * CW : (c + 1) * CW], in0=pt[0:64, :],
                    scalar1=bqk_sb[0:64, 0:1], scalar2=None, op0=add,
                )
                nc.scalar.activation(
                    out=qt_sb[w][:, 1, c * CW : (c + 1) * CW], in_=pt[64:128, :],
                    func=IDENT, bias=bqk_sb[64:128, 0:1], scale=1.0,
                )

            with tc.tile_pool(name="v_psum", bufs=2, space="PSUM") as v_psum:

                def vt_head(r):
                    # returns the psum tile; 8 accumulation matmuls issued via vt_mm
                    return v_psum.tile([QK, CW], F32, tag="v", name=f"vtp{r}")

                def vt_mm(pt, r, dt):
                    nc.tensor.matmul(
                        pt, lhsT=wv_sb[:, dt], rhs=plm_sb[r][:, dt],
                        start=(dt == 0), stop=(dt == 7),
                    )

                def vt_drain(pt, r):
                    nc.vector.tensor_scalar(
                        out=vt_sb[r], in0=pt, scalar1=bv_sb[:, 0:1],
                        scalar2=None, op0=add,
                    )
                    nc.gpsimd.dma_start(out=vt_out[r][:, :], in_=vt_sb[r])

                # ---- projections: KT, QT (DoubleRow fp8), VT chunks 0..wpc-1 ----
                with (
                    tc.tile_pool(name="kq_psum", bufs=4, space="PSUM") as kq_psum,
                    tc.tile_pool(name="t_psum", bufs=2, space="PSUM") as t_psum,
                ):
                    for w in range(wpc):
                        pt = kq_psum.tile([P, CW], F32, tag="kq")
                        n = 0
                        for t in range(5):
                            for g in range(2):
                                nc.tensor.matmul(
                                    pt,
                                    lhsT=wk_sb[:, t, g],
                                    rhs=evw_sb[w][g][:, :, t : t + CW],
                                    start=(n == 0),
                                    stop=(n == 9),
                                    perf_mode=DR,
                                )
                                n += 1
                        kt_split(w, pt)
                    for w in range(wpc):
                        for c in range(NCHUNK):
                            pt = kq_psum.tile([P, CW], F32, tag="kq")
                            for g in range(2):
                                nc.tensor.matmul(
                                    pt,
                                    lhsT=wq_sb[:, g],
                                    rhs=evq_sb[w][g][:, :, c * CW : (c + 1) * CW],
                                    start=(g == 0),
                                    stop=(g == 1),
                                    perf_mode=DR,
                                )
                            qt_split(w, c, pt)
                    # VT chunks 0..wpc-1 (feed V1) + their transposes
                    for r in range(min(wpc, NCHUNK)):
                        pt = vt_head(r)
                        for dt in range(8):
                            vt_mm(pt, r, dt)
                        vt_drain(pt, r)
                        for j in range(WT):
                            vp = t_psum.tile([P, QK], F16, tag="t")
                            nc.tensor.transpose(
                                vp, vt_sb[r][:, j * P : (j + 1) * P], ident_sb
                            )
                            nc.vector.tensor_copy(
                                out=v1_sb[r][:, j // 2, j % 2, :QK], in_=vp
                            )

                # ---- attention. Scalar exp ACTs pace the pipeline; between
                # every score unit the PE gets 2 filler matmuls (VT chunks
                # wpc..3 and dribbled OT accumulations) so it never idles
                # and holds max p-state. ----
                fills = []
                for r in range(wpc, NCHUNK):
                    holder = {}

                    def mk(r, dt, holder):
                        def run():
                            if dt == 0:
                                holder["pt"] = vt_head(r)
                            vt_mm(holder["pt"], r, dt)
                            if dt == 7:
                                vt_drain(holder["pt"], r)
                        return run

                    for dt in range(8):
                        fills.append(mk(r, dt, holder))
                fills.reverse()  # pop() from the front order

                def fill_one():
                    if fills:
                        fills.pop()()

                with (
                    tc.tile_pool(name="st_psum", bufs=2, space="PSUM") as st_psum,
                    tc.tile_pool(name="ot_psum", bufs=1, space="PSUM") as ot_psum,
                ):
                    ot_tiles = {}

                    def aunit(w, j, h):
                        stp = st_psum.tile([P, 2 * CW], F32, tag="st")
                        for o in range(2):
                            nc.tensor.matmul(
                                stp[:, o * CW : (o + 1) * CW],
                                lhsT=kt_sb[w][:, :, j * P : (j + 1) * P],
                                rhs=qt_sb[w][
                                    :, :, (2 * h + o) * CW : (2 * h + o + 1) * CW
                                ],
                                start=True,
                                stop=True,
                                perf_mode=DR,
                            )
                        nc.scalar.activation(
                            out=et_sb[w][:, j // 2, j % 2, h * 2 * CW : (h + 1) * 2 * CW],
                            in_=stp,
                            func=EXP,
                            bias=mask_sb[:, w * WT + j : w * WT + j + 1],
                            scale=NORM,
                        )

                    def ot_mm(w, h, p):
                        # one OT pair-accumulation step (2 matmuls: chunks of half h)
                        if (w, h) not in ot_tiles:
                            ot_tiles[(w, h)] = ot_psum.tile(
                                [112, 2 * CW], F32, tag="ot", name=f"otp{w}_{h}"
                            )
                        otp = ot_tiles[(w, h)]
                        for o in range(2):
                            c = 2 * h + o
                            nc.tensor.matmul(
                                otp[:, o * CW : (o + 1) * CW],
                                lhsT=v1_sb[w][:, p],
                                rhs=et_sb[w][:, p, :, c * CW : (c + 1) * CW],
                                start=(p == 0),
                                stop=(p == 1),
                                perf_mode=DR,
                            )

                    def ot_drain(w, h):
                        otp = ot_tiles.pop((w, h))
                        nc.vector.tensor_copy(
                            out=ot_sb[w][:, h * 2 * CW : (h + 1) * 2 * CW],
                            in_=otp[: QK + 1, :],
                        )
                        for s in range(4):
                            c0 = h * 2 * CW + s * 2 * P
                            nc.sync.dma_start(
                                out=ot_out[w][:QK, c0 : c0 + 2 * P],
                                in_=ot_sb[w][:QK, c0 : c0 + 2 * P],
                            )
                        nc.gpsimd.dma_start(
                            out=ot_out[w][QK : QK + 1, h * 2 * CW : (h + 1) * 2 * CW],
                            in_=ot_sb[w][QK : QK + 1, h * 2 * CW : (h + 1) * 2 * CW],
                        )

                    # per-window unit stream (j,h) with slotted fillers:
                    #   u3 -> OT(h0,p0); u5 -> OT(h0,p1)+drain; u6 -> OT(h1,p0)
                    #   next window's u0 (or post-loop) -> OT(h1,p1)+drain
                    UNITS = [(0, 0), (1, 0), (2, 0), (3, 0), (0, 1), (1, 1), (2, 1), (3, 1)]
                    carry = None
                    for w in range(wpc):
                        for u, (j, h) in enumerate(UNITS):
                            aunit(w, j, h)
                            if u == 0 and carry is not None:
                                ot_mm(*carry, 1)
                                ot_drain(*carry)
                                carry = None
                            elif u == 3:
                                ot_mm(w, 0, 0)
                            elif u == 5:
                                ot_mm(w, 0, 1)
                                ot_drain(w, 0)
                            elif u == 6:
                                ot_mm(w, 1, 0)
                            else:
                                fill_one()
                                fill_one()
                        carry = (w, 1)
                    if carry is not None:
                        ot_mm(*carry, 1)
                        ot_drain(*carry)
                    while fills:
                        fills.pop()()
    nc.finalize()
    return nc


def _pack_pair_w(w, nk):
    """(nk*128, M) f32 -> [128, nk/2, 2, M] DoubleRow pair layout."""
    kt, m = nk, w.shape[1]
    v = w.reshape(kt, P, m).reshape(kt // 2, 2, P, m).transpose(2, 0, 1, 3)
    return np.ascontiguousarray(v)


def _plan(seqlengths):
    """Assign windows + residual chunks to cores."""
    nkt = [max(1, min(L // P, -(-int(s) // P))) for s in seqlengths]
    wins = [(b, w) for b in range(B) for w in range(-(-nkt[b] // WT))]
    wpc = max(1, -(-len(wins) // B))
    wins = wins + [None] * (B * wpc - len(wins))
    windows = [wins[c * wpc : (c + 1) * wpc] for c in range(B)]
    # R slot w must hold window w's key-column chunk (V1 derives locally)
    all_chunks = {(b, r) for b in range(B) for r in range(NCHUNK)}
    chunks = [[None] * NCHUNK for _ in range(B)]
    for c in range(B):
        for w, bw in enumerate(windows[c]):
            if w < NCHUNK and bw is not None:
                assert bw in all_chunks
                chunks[c][w] = bw
                all_chunks.discard(bw)
    rest = sorted(all_chunks)
    for c in range(B):
        for r in range(NCHUNK):
            if chunks[c][r] is None:
                chunks[c][r] = rest.pop()
    assert not rest
    return wpc, nkt, windows, chunks


def _prep_core(core, wpc, nkt, windows, chunks, evoT8, plmT, seqlengths, weights):
    m = dict(weights)
    mask = np.full((P, wpc * WT), -1e6, np.float32)
    p = np.arange(P)
    for w, bw in enumerate(windows[core]):
        if bw is None:
            m[f"evoW{w}"] = np.zeros((P, 4 * (CW + 4)), NP8)
            m[f"evoQ{w}"] = np.zeros((P, 4 * L), NP8)
            continue
        b, wi = bw
        sl = int(seqlengths[b])
        base = wi * WT * P
        for j in range(WT):
            mask[:, w * WT + j] = np.where(base + j * P + p < sl, 0.0, -1e6)
        sl_ = evoT8[b][:, base : base + CW + 4]
        m[f"evoW{w}"] = np.ascontiguousarray(
            sl_.reshape(4, P, CW + 4).transpose(1, 0, 2).reshape(P, -1)
        )
        m[f"evoQ{w}"] = np.ascontiguousarray(
            evoT8[b][:, 2 : 2 + L].reshape(4, P, L).transpose(1, 0, 2).reshape(P, -1)
        )
    m["mask"] = mask
    for r, (b, rc) in enumerate(chunks[core]):
        sl_ = plmT[b][:, rc * CW : (rc + 1) * CW]
        m[f"plm{r}"] = np.ascontiguousarray(
            sl_.reshape(8, P, CW).transpose(1, 0, 2).reshape(P, -1)
        )
    return m


def kernel(
    plm_embedding,
    evo_local,
    seqlengths,
    q_w,
    q_b,
    k_w,
    k_b,
    v_w,
    v_b,
    cn3_w,
    cn3_b,
    cn5_w,
    cn5_b,
):
    global LAST_EXEC_TIME_NS, LAST_RESULTS
    plm_embedding = np.asarray(plm_embedding, np.float32)
    evo_local = np.asarray(evo_local, np.float32)
    seqlengths = np.asarray(seqlengths)

    taps, bk = _fold_k_weights(
        np.asarray(k_w, np.float32),
        np.asarray(k_b, np.float32),
        np.asarray(cn3_w, np.float32),
        np.asarray(cn3_b, np.float32),
        np.asarray(cn5_w, np.float32),
        np.asarray(cn5_b, np.float32),
    )
    wpc, nkt, windows, chunks = _plan(seqlengths)

    # fp8 weights, M padded 96 -> 128 with zeros (pad rows of Q/K then
    # contribute exactly 0 to scores; biases pad with zeros too)
    wq_p = np.zeros((Q_IN, P), np.float32)
    wq_p[:, :QK] = np.asarray(q_w, np.float32).T
    wk_p = np.zeros((5 * Q_IN, P), np.float32)
    wk_p[:, :QK] = taps.reshape(5 * Q_IN, QK)
    bqk = np.zeros((P, 2), np.float32)
    bqk[:QK, 0] = np.asarray(q_b, np.float32)
    bqk[:QK, 1] = bk
    weights = {
        "wq": np.ascontiguousarray(_pack_pair_w(wq_p, 4).reshape(P, -1)).astype(NP8),
        "wk": np.ascontiguousarray(_pack_pair_w(wk_p, 20).reshape(P, -1)).astype(NP8),
        "wv": np.ascontiguousarray(
            np.asarray(v_w, np.float32)
            .T.reshape(8, P, QK)
            .transpose(1, 0, 2)
            .reshape(P, -1)
        ).astype(np.float16),
        "bqk": bqk,
        "bv": np.ascontiguousarray(np.asarray(v_b, np.float32)[:, None]),
        "ident": np.eye(QK, dtype=np.float16),
    }

    evoT8 = np.zeros((B, Q_IN, L + 4), NP8)
    evoT8[:, :, 2 : 2 + L] = np.clip(
        evo_local.transpose(0, 2, 1), -240.0, 240.0
    ).astype(NP8)
    plmT = plm_embedding.transpose(0, 2, 1).astype(np.float16)

    if wpc not in _program_cache:
        _program_cache[wpc] = _build_program(wpc)
    nc = _program_cache[wpc]

    in_maps = [
        _prep_core(c, wpc, nkt, windows, chunks, evoT8, plmT, seqlengths, weights)
        for c in range(B)
    ]
    trace = bool(os.environ.get("KBENCH_TRACE"))
    res = run_bass_kernel_spmd(nc, in_maps, list(range(B)), trace=trace)
    LAST_EXEC_TIME_NS = res.exec_time_ns
    LAST_RESULTS = res

    num = np.zeros((B, QK, L), np.float32)
    den = np.zeros((B, 1, L), np.float32)
    vt = np.zeros((B, QK, L), np.float32)
    for c in range(B):
        for w, bw in enumerate(windows[c]):
            if bw is None:
                continue
            b, _ = bw
            ot = res.results[c][f"ot{w}"]
            num[b] += ot[:QK]
            den[b] += ot[QK : QK + 1]
        for r, (b, rc) in enumerate(chunks[c]):
            vt[b][:, rc * CW : (rc + 1) * CW] = res.results[c][f"vt{r}"]
    out = ((num / den) + vt).transpose(0, 2, 1).astype(np.float32)
    return np.ascontiguousarray(out)


# revision 29
# speedup vs baseline: 1.0698x; 1.0629x over previous
"""Contextual-attention Trainium2 kernel (Bass/Tile), work-balanced across cores.

Math (per sequence b):
    Q = evo @ q_w.T + q_b                                  (L, 96)
    K = cat(evo, conv3(evo), conv5(evo)) @ k_w.T + k_b     (L, 96)
    V = plm @ v_w.T + v_b                                  (L, 96)
    P = softmax(Q K^T / sqrt(96), key-masked by seqlen)
    out = P @ V + V

Design notes (measured on hw):
  * PE streams 1 output column/cycle (~1.35 GHz) regardless of dtype; fp8
    DoubleRow processes 2 k-tiles per pass -> 2x for pairable contractions
    (QT/KT/OT). The 96-dim score contraction is not pairable beyond a
    zero-padded 2x64 split, so ST cost is fixed; balancing it across cores
    is what wins.
  * Work per sequence scales with nkt_b = ceil(seqlen_b/128) key tiles
    (2..14 here). Attention is split into 4-key-tile "windows" (512 key
    cols x all 2048 queries); every core gets exactly wpc windows + 4 fp16
    V chunks -> perfectly uniform SPMD program; host sums the partial
    (numerator|denominator) window outputs.
  * The PE drops to a ~1.66x slower p-state after any blocking wait and
    needs ~3us of uninterrupted issue to recover, so the emission order
    keeps every PE instruction's deps satisfied ahead of time: VT chunks
    2..3 are interleaved as filler between ACT-bound score units, OT
    drains per half-window, DMA issue is spread across sync/gpsimd
    queues, and the scalar queue carries nothing but the exp ACTs.
  * Masking: per (window-tile) per-partition bias 0/-1e6 into the exp ACT
    reproduces the reference where()+softmax exactly (exp(-1e6+s) == 0).
"""

import os
import numpy as np
import ml_dtypes

import concourse.bacc as bacc
import concourse.bass as bass
import concourse.tile as tile
from concourse import mybir
from concourse._compat import get_trn_type
from concourse.bass_utils import run_bass_kernel_spmd

B, L = 8, 2048
Q_IN, V_IN, QK, VD = 512, 1024, 96, 96
P = 128
CW = 512          # column chunk width (= one PSUM bank of f32)
WT = 4            # key tiles per window
NCHUNK = L // CW  # 4 column chunks per sequence
NORM = float(1.0 / np.sqrt(QK))
F32 = mybir.dt.float32
F16 = mybir.dt.float16
F8 = mybir.dt.float8e4
NP8 = ml_dtypes.float8_e4m3
DR = mybir.MatmulPerfMode.DoubleRow
EXP = mybir.ActivationFunctionType.Exp

LAST_EXEC_TIME_NS = None
LAST_RESULTS = None

_program_cache = {}


def _fold_k_weights(k_w, k_b, cn3_w, cn3_b, cn5_w, cn5_b):
    """K[l] = sum_{t in -2..2} evo[l+t] @ taps[t+2] + bk  (zero-padded shifts)."""
    A_evo = k_w[:, :Q_IN]
    A3 = k_w[:, Q_IN : Q_IN + VD]
    A5 = k_w[:, Q_IN + VD :]
    taps = np.zeros((5, Q_IN, QK), np.float32)
    for j in range(3):  # conv3 tap j acts at offset t = j-1
        taps[j - 1 + 2] += np.einsum("oc,cd->do", A3, cn3_w[:, :, j]).astype(np.float32)
    for j in range(5):  # conv5 tap j acts at offset t = j-2
        taps[j - 2 + 2] += np.einsum("oc,cd->do", A5, cn5_w[:, :, j]).astype(np.float32)
    taps[2] += A_evo.T
    bk = (k_b + A3 @ cn3_b + A5 @ cn5_b).astype(np.float32)
    return taps, bk


def _build_program(wpc):
    nc = bacc.Bacc(get_trn_type() or "TRN2", target_bir_lowering=False, debug=False)
    wq = nc.declare_dram_parameter("wq", [P, 2 * 2 * P], F8, isOutput=False)
    wk = nc.declare_dram_parameter("wk", [P, 5 * 2 * 2 * P], F8, isOutput=False)
    wv = nc.declare_dram_parameter("wv", [P, 8 * QK], F16, isOutput=False)
    bqk = nc.declare_dram_parameter("bqk", [P, 2], F32, isOutput=False)
    bv = nc.declare_dram_parameter("bv", [QK, 1], F32, isOutput=False)
    maskd = nc.declare_dram_parameter("mask", [P, wpc * WT], F32, isOutput=False)
    identd = nc.declare_dram_parameter("ident", [QK, QK], F16, isOutput=False)
    evoW = [
        nc.declare_dram_parameter(f"evoW{w}", [P, 4 * (CW + 4)], F8, isOutput=False)
        for w in range(wpc)
    ]
    evoQ = [
        nc.declare_dram_parameter(f"evoQ{w}", [P, 4 * L], F8, isOutput=False)
        for w in range(wpc)
    ]
    plm = [
        nc.declare_dram_parameter(f"plm{r}", [P, 8 * CW], F16, isOutput=False)
        for r in range(NCHUNK)
    ]
    ot_out = [
        nc.declare_dram_parameter(f"ot{w}", [QK + 1, L], F16, isOutput=True)
        for w in range(wpc)
    ]
    vt_out = [
        nc.declare_dram_parameter(f"vt{r}", [QK, CW], F16, isOutput=True)
        for r in range(NCHUNK)
    ]

    add = mybir.AluOpType.add
    IDENT = mybir.ActivationFunctionType.Identity

    with tile.TileContext(nc) as tc:
        with tc.tile_pool(name="sing", bufs=1) as sing:
            # ---- SBUF tiles ----
            wq_sb = sing.tile([P, 2, 2, P], F8, tag="wq")
            wk_sb = sing.tile([P, 5, 2, 2, P], F8, tag="wk")
            wv_sb = sing.tile([P, 8, QK], F16, tag="wv")
            bqk_sb = sing.tile([P, 2], F32, tag="bqk")
            bv_sb = sing.tile([QK, 1], F32, tag="bv")
            mask_sb = sing.tile([P, wpc * WT], F32, tag="mask")
            ident_sb = sing.tile([QK, QK], F16, tag="ident")
            evw_sb = [
                [sing.tile([P, 2, CW + 4], F8, tag=f"evw{w}_{g}", name=f"evw{w}_{g}")
                 for g in range(2)]
                for w in range(wpc)
            ]
            evq_sb = [
                [sing.tile([P, 2, L], F8, tag=f"evq{w}_{g}", name=f"evq{w}_{g}")
                 for g in range(2)]
                for w in range(wpc)
            ]
            plm_sb = [
                sing.tile([P, 8, CW], F16, tag=f"plm{r}", name=f"plmsb{r}")
                for r in range(NCHUNK)
            ]
            kt_sb = [sing.tile([64, 2, CW], F8, tag=f"kt{w}", name=f"kt{w}") for w in range(wpc)]
            qt_sb = [sing.tile([64, 2, L], F8, tag=f"qt{w}", name=f"qt{w}") for w in range(wpc)]
            vt_sb = [sing.tile([QK, CW], F16, tag=f"vt{r}", name=f"vt{r}") for r in range(NCHUNK)]
            v1_sb = [sing.tile([P, 2, 2, 112], F8, tag=f"v1_{w}", name=f"v1_{w}") for w in range(wpc)]
            et_sb = [sing.tile([P, 2, 2, L], F8, tag=f"et{w}", name=f"et{w}") for w in range(wpc)]
            ot_sb = [sing.tile([QK + 1, L], F16, tag=f"ot{w}", name=f"ot{w}") for w in range(wpc)]

            # ---- DMA issue. Small critical loads first on every queue, the
            # 4MB plm flood behind them, split x8 per chunk. ----
            nc.sync.dma_start(
                out=wq_sb, in_=wq[:, :].rearrange("p (g i m) -> p g i m", g=2, i=2)
            )
            wk_r = wk[:, :].rearrange("p (t g i m) -> p t g i m", t=5, g=2, i=2)
            for t4 in range(5):
                nc.sync.dma_start(out=wk_sb[:, t4], in_=wk_r[:, t4])
            for w in range(wpc):
                for g in range(2):
                    nc.gpsimd.dma_start(
                        out=evw_sb[w][g],
                        in_=evoW[w][:, :].rearrange("p (g i c) -> p g i c", g=2, i=2)[:, g],
                    )
            # evq w0 split across sync+gpsimd queue heads (QT w0 gates the
            # pipeline start); evq w1 follows on sync. Scalar queue stays
            # clear for the exp ACTs (DMA issues are credit-throttled).
            for g in range(2):
                eng = nc.sync if g == 0 else nc.gpsimd
                for h in range(2):
                    for q in range(4):
                        eng.dma_start(
                            out=evq_sb[0][g][h * 64 : (h + 1) * 64, :, q * 512 : (q + 1) * 512],
                            in_=evoQ[0][h * 64 : (h + 1) * 64, :]
                            .rearrange("p (g i c) -> p g i c", g=2, i=2)[:, g, :, q * 512 : (q + 1) * 512],
                        )
            for w in range(1, wpc):
                for g in range(2):
                    for h in range(2):
                        for q in range(2):
                            nc.sync.dma_start(
                                out=evq_sb[w][g][h * 64 : (h + 1) * 64, :, q * 1024 : (q + 1) * 1024],
                                in_=evoQ[w][h * 64 : (h + 1) * 64, :]
                                .rearrange("p (g i c) -> p g i c", g=2, i=2)[:, g, :, q * 1024 : (q + 1) * 1024],
                            )

            def plm_issue(eng, r):
                for n in range(8):
                    eng.dma_start(
                        out=plm_sb[r][:, n : n + 1, :],
                        in_=plm[r][:, :].rearrange("p (n c) -> p n c", n=8)[:, n : n + 1],
                    )

            nc.gpsimd.dma_start(out=bqk_sb, in_=bqk[:, :])
            nc.gpsimd.dma_start(out=mask_sb, in_=maskd[:, :])
            nc.gpsimd.dma_start(
                out=wv_sb, in_=wv[:, :].rearrange("p (n o) -> p n o", o=QK)
            )
            nc.gpsimd.dma_start(out=bv_sb, in_=bv[:, :])
            nc.gpsimd.dma_start(out=ident_sb, in_=identd[:, :])
            plm_issue(nc.gpsimd, 0)
            if NCHUNK > 1:
                plm_issue(nc.gpsimd, 1)
            if NCHUNK > 2:
                plm_issue(nc.sync, 2)
            if NCHUNK > 3:
                plm_issue(nc.sync, 3)

            for w in range(wpc):
                nc.vector.memset(v1_sb[w], 0.0)
                for p in range(2):
                    for i in range(2):
                        nc.vector.memset(v1_sb[w][:, p, i, QK : QK + 1], 1.0)

            # split helpers: during the head phase scalar is free (i=1 there);
            # during attention both halves go to vector (scalar paces exp).
            def kt_split(w, pt, att):
                nc.vector.tensor_scalar(
                    out=kt_sb[w][:, 0, :], in0=pt[0:64, :],
                    scalar1=bqk_sb[0:64, 1:2], scalar2=None, op0=add,
                )
                if att:
                    nc.vector.tensor_scalar(
                        out=kt_sb[w][:, 1, :], in0=pt[64:128, :],
                        scalar1=bqk_sb[64:128, 1:2], scalar2=None, op0=add,
                    )
                else:
                    nc.scalar.activation(
                        out=kt_sb[w][:, 1, :], in_=pt[64:128, :],
                        func=IDENT, bias=bqk_sb[64:128, 1:2], scale=1.0,
                    )

            def qt_split(w, c, pt, att):
                nc.vector.tensor_scalar(
                    out=qt_sb[w][:, 0, c * CW : (c + 1) * CW], in0=pt[0:64, :],
                    scalar1=bqk_sb[0:64, 0:1], scalar2=None, op0=add,
                )
                if att:
                    nc.vector.tensor_scalar(
                        out=qt_sb[w][:, 1, c * CW : (c + 1) * CW], in0=pt[64:128, :],
                        scalar1=bqk_sb[64:128, 0:1], scalar2=None, op0=add,
                    )
                else:
                    nc.scalar.activation(
                        out=qt_sb[w][:, 1, c * CW : (c + 1) * CW], in_=pt[64:128, :],
                        func=IDENT, bias=bqk_sb[64:128, 0:1], scale=1.0,
                    )

            with (
                tc.tile_pool(name="proj", bufs=2, space="PSUM") as proj,
                tc.tile_pool(name="st_psum", bufs=2, space="PSUM") as st_psum,
                tc.tile_pool(name="ot_psum", bufs=1, space="PSUM") as ot_psum,
            ):
                def kt_mms(w):
                    pt = proj.tile([P, CW], F32, tag="proj", name=f"ktp{w}")
                    out = []
                    n = 0
                    for t in range(5):
                        for g in range(2):
                            out.append((pt, t, g, n == 0, n == 9))
                            n += 1
                    return pt, out

                def emit_kt_mm(w, pt, t, g, st, sp):
                    nc.tensor.matmul(
                        pt, lhsT=wk_sb[:, t, g],
                        rhs=evw_sb[w][g][:, :, t : t + CW],
                        start=st, stop=sp, perf_mode=DR,
                    )

                def emit_qt_mm(w, c, pt, g):
                    nc.tensor.matmul(
                        pt, lhsT=wq_sb[:, g],
                        rhs=evq_sb[w][g][:, :, c * CW : (c + 1) * CW],
                        start=(g == 0), stop=(g == 1), perf_mode=DR,
                    )

                def emit_vt_mm(r, pt, dt):
                    nc.tensor.matmul(
                        pt[:QK, :], lhsT=wv_sb[:, dt], rhs=plm_sb[r][:, dt],
                        start=(dt == 0), stop=(dt == 7),
                    )

                def vt_finish(r, pt, w=None):
                    # bias -> vt chunk, DMA out; if it feeds window w, also
                    # PE-transpose (into an st-pool tile via f16 bitcast) and
                    # cast into V1.
                    nc.vector.tensor_scalar(
                        out=vt_sb[r], in0=pt[:QK, :], scalar1=bv_sb[:, 0:1],
                        scalar2=None, op0=add,
                    )
                    nc.gpsimd.dma_start(out=vt_out[r][:, :], in_=vt_sb[r])
                    if w is not None:
                        for j in range(WT):
                            tt = st_psum.tile(
                                [P, 2 * CW], F32, tag="st", name=f"tp{w}_{j}"
                            )
                            t16 = tt.bitcast(F16)[:, :QK]
                            nc.tensor.transpose(
                                t16, vt_sb[r][:, j * P : (j + 1) * P], ident_sb
                            )
                            nc.vector.tensor_copy(
                                out=v1_sb[w][:, j // 2, j % 2, :QK], in_=t16
                            )

                # ---- head: window-0 projections ----
                ktp0, kt_list = kt_mms(0)
                for (pt, t, g, st, sp) in kt_list:
                    emit_kt_mm(0, pt, t, g, st, sp)
                kt_split(0, ktp0, att=False)
                for c in range(NCHUNK):
                    pt = proj.tile([P, CW], F32, tag="proj", name=f"qtp0_{c}")
                    for g in range(2):
                        emit_qt_mm(0, c, pt, g)
                    qt_split(0, c, pt, att=False)
                # ---- filler queue: remaining projections as closures.
                # Each entry: (is_mm, fn). Deadline markers gate window w's
                # KT/QT completion before its first score unit. ----
                fillers = []
                deadlines = {}
                for w in range(1, wpc):
                    ktp, kt_list = kt_mms(w)
                    for (pt, t, g, st, sp) in kt_list:
                        fillers.append((lambda w=w, pt=pt, t=t, g=g, st=st, sp=sp:
                                        emit_kt_mm(w, pt, t, g, st, sp)))
                    fillers.append(lambda w=w, pt=ktp: kt_split(w, pt, att=True))
                    for c in range(NCHUNK):
                        holder = {}

                        def qt_group(w=w, c=c, holder=holder):
                            holder["pt"] = proj.tile(
                                [P, CW], F32, tag="proj", name=f"qtp{w}_{c}"
                            )
                        fillers.append(qt_group)
                        for g in range(2):
                            fillers.append(lambda w=w, c=c, g=g, holder=holder:
                                           emit_qt_mm(w, c, holder["pt"], g))
                        fillers.append(lambda w=w, c=c, holder=holder:
                                       qt_split(w, c, holder["pt"], att=True))
                    deadlines.setdefault(8 * w, [None])[0] = len(fillers)
                n_stream_a = len(fillers)
                for r in range(NCHUNK):
                    holder = {}

                    def vt_group(r=r, holder=holder):
                        holder["pt"] = proj.tile(
                            [P, CW], F32, tag="proj", name=f"vtp{r}"
                        )
                    fillers.append(vt_group)
                    for dt in range(8):
                        fillers.append(lambda r=r, dt=dt, holder=holder:
                                       emit_vt_mm(r, holder["pt"], dt))
                    wref = r if r < wpc else None
                    fillers.append(lambda r=r, holder=holder, wref=wref:
                                   vt_finish(r, holder["pt"], w=wref))
                    if r < wpc:
                        # V1 of window r is needed by its first OT single
                        deadlines.setdefault(8 + 4 * r, [None])[0] = len(fillers)
                deadlines = {k: v[0] for k, v in deadlines.items()}

                fi = {"i": 0}

                def run_fillers(upto):
                    while fi["i"] < upto:
                        fillers[fi["i"]]()
                        fi["i"] += 1

                # ---- attention units + woven OT singles + paced fillers ----
                ot_tiles = {}

                def aunit(w, j, h):
                    stp = st_psum.tile([P, 2 * CW], F32, tag="st")
                    for o in range(2):
                        nc.tensor.matmul(
                            stp[:, o * CW : (o + 1) * CW],
                            lhsT=kt_sb[w][:, :, j * P : (j + 1) * P],
                            rhs=qt_sb[w][:, :, (2 * h + o) * CW : (2 * h + o + 1) * CW],
                            start=True, stop=True, perf_mode=DR,
                        )
                    nc.scalar.activation(
                        out=et_sb[w][:, j // 2, j % 2, h * 2 * CW : (h + 1) * 2 * CW],
                        in_=stp, func=EXP,
                        bias=mask_sb[:, w * WT + j : w * WT + j + 1],
                        scale=NORM,
                    )

                def ot_single(w, h, p, o):
                    if (w, h) not in ot_tiles:
                        ot_tiles[(w, h)] = ot_psum.tile(
                            [112, 2 * CW], F32, tag="ot", name=f"otp{w}_{h}"
                        )
                    otp = ot_tiles[(w, h)]
                    c = 2 * h + o
                    nc.tensor.matmul(
                        otp[:, o * CW : (o + 1) * CW],
                        lhsT=v1_sb[w][:, p],
                        rhs=et_sb[w][:, p, :, c * CW : (c + 1) * CW],
                        start=(p == 0), stop=(p == 1), perf_mode=DR,
                    )

                def ot_drain(w, h):
                    otp = ot_tiles.pop((w, h))
                    nc.vector.tensor_copy(
                        out=ot_sb[w][:, h * 2 * CW : (h + 1) * 2 * CW],
                        in_=otp[: QK + 1, :],
                    )
                    for s in range(8):
                        c0 = h * 2 * CW + s * P
                        eng = nc.sync if s % 2 == 0 else nc.gpsimd
                        eng.dma_start(
                            out=ot_out[w][:QK, c0 : c0 + P],
                            in_=ot_sb[w][:QK, c0 : c0 + P],
                        )
                    nc.gpsimd.dma_start(
                        out=ot_out[w][QK : QK + 1, h * 2 * CW : (h + 1) * 2 * CW],
                        in_=ot_sb[w][QK : QK + 1, h * 2 * CW : (h + 1) * 2 * CW],
                    )

                units = [(w, j, h) for w in range(wpc) for h in range(2) for j in range(WT)]
                singles = [
                    (w, h, p, o)
                    for w in range(wpc)
                    for h in range(2)
                    for p in range(2)
                    for o in range(2)
                ]
                drains = {}
                for w in range(wpc):
                    for h in range(2):
                        drains[w * 8 + h * 4 + 3] = (w, h)
                U, F = len(units), len(fillers)
                # singles must come after the VT r0 fillers on the in-order
                # PE queue (v1 dependency); unit 8 is gated by the deadline.
                S0 = 8
                # two-stream pacing: evo-only fillers (KT/QT of later windows)
                # over units 0..5; plm-dependent VT fillers over units 6..U-1
                # (plm chunks only land ~2/3 through the input stream).
                GA = min(6, U)
                FB = F - n_stream_a

                def quota(g):
                    if g < GA:
                        return (n_stream_a * (g + 1) + GA - 1) // GA
                    return n_stream_a + (FB * (g - GA + 1) + (U - GA) - 1) // (U - GA)

                si = 0
                for g, u in enumerate(units):
                    if g in deadlines:
                        run_fillers(deadlines[g])
                    aunit(*u)
                    if g >= S0:
                        for _ in range(2):
                            if si < len(singles):
                                ot_single(*singles[si])
                                if si in drains:
                                    ot_drain(*drains[si])
                                si += 1
                    run_fillers(min(F, quota(g)))
                run_fillers(F)
                while si < len(singles):
                    ot_single(*singles[si])
                    if si in drains:
                        ot_drain(*drains[si])
                    si += 1
    nc.finalize()
    return nc


def _pack_pair_w(w, nk):
    """(nk*128, M) f32 -> [128, nk/2, 2, M] DoubleRow pair layout."""
    kt, m = nk, w.shape[1]
    v = w.reshape(kt, P, m).reshape(kt // 2, 2, P, m).transpose(2, 0, 1, 3)
    return np.ascontiguousarray(v)


def _plan(seqlengths):
    """Assign windows + residual chunks to cores."""
    nkt = [max(1, min(L // P, -(-int(s) // P))) for s in seqlengths]
    wins = [(b, w) for b in range(B) for w in range(-(-nkt[b] // WT))]
    wpc = max(1, -(-len(wins) // B))
    wins = wins + [None] * (B * wpc - len(wins))
    windows = [wins[c * wpc : (c + 1) * wpc] for c in range(B)]
    # R slot w must hold window w's key-column chunk (V1 derives locally)
    all_chunks = {(b, r) for b in range(B) for r in range(NCHUNK)}
    chunks = [[None] * NCHUNK for _ in range(B)]
    for c in range(B):
        for w, bw in enumerate(windows[c]):
            if w < NCHUNK and bw is not None:
                assert bw in all_chunks
                chunks[c][w] = bw
                all_chunks.discard(bw)
    rest = sorted(all_chunks)
    for c in range(B):
        for r in range(NCHUNK):
            if chunks[c][r] is None:
                chunks[c][r] = rest.pop()
    assert not rest
    return wpc, nkt, windows, chunks


def _prep_core(core, wpc, nkt, windows, chunks, evoT8, plmT, seqlengths, weights):
    m = dict(weights)
    mask = np.full((P, wpc * WT), -1e6, np.float32)
    p = np.arange(P)
    for w, bw in enumerate(windows[core]):
        if bw is None:
            m[f"evoW{w}"] = np.zeros((P, 4 * (CW + 4)), NP8)
            m[f"evoQ{w}"] = np.zeros((P, 4 * L), NP8)
            continue
        b, wi = bw
        sl = int(seqlengths[b])
        base = wi * WT * P
        for j in range(WT):
            mask[:, w * WT + j] = np.where(base + j * P + p < sl, 0.0, -1e6)
        sl_ = evoT8[b][:, base : base + CW + 4]
        m[f"evoW{w}"] = np.ascontiguousarray(
            sl_.reshape(4, P, CW + 4).transpose(1, 0, 2).reshape(P, -1)
        )
        m[f"evoQ{w}"] = np.ascontiguousarray(
            evoT8[b][:, 2 : 2 + L].reshape(4, P, L).transpose(1, 0, 2).reshape(P, -1)
        )
    m["mask"] = mask
    for r, (b, rc) in enumerate(chunks[core]):
        sl_ = plmT[b][:, rc * CW : (rc + 1) * CW]
        m[f"plm{r}"] = np.ascontiguousarray(
            sl_.reshape(8, P, CW).transpose(1, 0, 2).reshape(P, -1)
        )
    return m


def kernel(
    plm_embedding,
    evo_local,
    seqlengths,
    q_w,
    q_b,
    k_w,
    k_b,
    v_w,
    v_b,
    cn3_w,
    cn3_b,
    cn5_w,
    cn5_b,
):
    global LAST_EXEC_TIME_NS, LAST_RESULTS
    plm_embedding = np.asarray(plm_embedding, np.float32)
    evo_local = np.asarray(evo_local, np.float32)
    seqlengths = np.asarray(seqlengths)

    taps, bk = _fold_k_weights(
        np.asarray(k_w, np.float32),
        np.asarray(k_b, np.float32),
        np.asarray(cn3_w, np.float32),
        np.asarray(cn3_b, np.float32),
        np.asarray(cn5_w, np.float32),
        np.asarray(cn5_b, np.float32),
    )
    wpc, nkt, windows, chunks = _plan(seqlengths)

    # fp8 weights, M padded 96 -> 128 with zeros (pad rows of Q/K then
    # contribute exactly 0 to scores; biases pad with zeros too)
    wq_p = np.zeros((Q_IN, P), np.float32)
    wq_p[:, :QK] = np.asarray(q_w, np.float32).T
    wk_p = np.zeros((5 * Q_IN, P), np.float32)
    wk_p[:, :QK] = taps.reshape(5 * Q_IN, QK)
    bqk = np.zeros((P, 2), np.float32)
    bqk[:QK, 0] = np.asarray(q_b, np.float32)
    bqk[:QK, 1] = bk
    weights = {
        "wq": np.ascontiguousarray(_pack_pair_w(wq_p, 4).reshape(P, -1)).astype(NP8),
        "wk": np.ascontiguousarray(_pack_pair_w(wk_p, 20).reshape(P, -1)).astype(NP8),
        "wv": np.ascontiguousarray(
            np.asarray(v_w, np.float32)
            .T.reshape(8, P, QK)
            .transpose(1, 0, 2)
            .reshape(P, -1)
        ).astype(np.float16),
        "bqk": bqk,
        "bv": np.ascontiguousarray(np.asarray(v_b, np.float32)[:, None]),
        "ident": np.eye(QK, dtype=np.float16),
    }

    evoT8 = np.zeros((B, Q_IN, L + 4), NP8)
    evoT8[:, :, 2 : 2 + L] = np.clip(
        evo_local.transpose(0, 2, 1), -240.0, 240.0
    ).astype(NP8)
    plmT = plm_embedding.transpose(0, 2, 1).astype(np.float16)

    if wpc not in _program_cache:
        _program_cache[wpc] = _build_program(wpc)
    nc = _program_cache[wpc]

    in_maps = [
        _prep_core(c, wpc, nkt, windows, chunks, evoT8, plmT, seqlengths, weights)
        for c in range(B)
    ]
    trace = bool(os.environ.get("KBENCH_TRACE"))
    res = run_bass_kernel_spmd(nc, in_maps, list(range(B)), trace=trace)
    LAST_EXEC_TIME_NS = res.exec_time_ns
    LAST_RESULTS = res

    num = np.zeros((B, QK, L), np.float32)
    den = np.zeros((B, 1, L), np.float32)
    vt = np.zeros((B, QK, L), np.float32)
    for c in range(B):
        for w, bw in enumerate(windows[c]):
            if bw is None:
                continue
            b, _ = bw
            ot = res.results[c][f"ot{w}"]
            num[b] += ot[:QK]
            den[b] += ot[QK : QK + 1]
        for r, (b, rc) in enumerate(chunks[c]):
            vt[b][:, rc * CW : (rc + 1) * CW] = res.results[c][f"vt{r}"]
    out = ((num / den) + vt).transpose(0, 2, 1).astype(np.float32)
    return np.ascontiguousarray(out)
